# revision 31
# baseline (speedup 1.0000x reference)
"""Weighted Chamfer loss on Trainium2 (8 NeuronCores, batch-parallel).

Problem (per batch element b of 8):
    dist[i, j] = || set1[b, i] - set2[b, j] ||_2            (4096 x 4096, C=128)
    total = (sum_i w1[b,i] * min_j dist + sum_j w2[b,j] * min_i dist) / 2

Sharding: one batch element per NeuronCore (pure data parallel, no
collectives); the 8 per-core partial sums are added on the host.

Per-core pipeline (units are [128 x 2048] fp32 PSUM tiles; 32 row blocks
x 2 column halves):
  PE  : ONE fp8 DoubleRow matmul per 512-col chunk computes
        m = x.y - x2/2 - y2/2 (= -d2/2) over a K=256 contraction:
        k-tile 0 = the 128 channels (e4m3-cast inputs), k-tile 1 = bake
        rows (3 fp8 residual rows per squared-norm + matching ones rows).
  exp mode (default): evacuation carries the row reduction.
    ACT : evacuates E-blocks as exp(-beta*(d2 - REF)) in bf16
          (Exp(scale=2*beta, bias=beta*REF) of the PSUM); its built-in
          sum accumulator emits per-unit softmin sums -> row-min comes
          out of the evac pass for free (rowmin = REF - ln(S)/beta).
    DVE : col pass only = tensor_tensor(max) of the (monotone) exps into
          colacc; for `dcopy` D-blocks DVE instead evacuates from PSUM
          via tensor_scalar(mult -2 -> fp16 d2) whose accum(min) is that
          half's row-min, and folds into a separate fp16 min-colacc
          (rebalances ACT vs DVE).
    Tail: PE transposes of both col accumulators + strided max/min
          reduces; Ln/Relu/Sqrt on ACT; masked merge of the two row/col
          conventions; weighted sums; host adds the 8 partials.
"""

import sys
from contextlib import ExitStack, nullcontext

import numpy as np

for _p in ("/opt/trn_rl_repo",):
    if _p not in sys.path:
        sys.path.insert(0, _p)

import concourse.bass as bass
import concourse.tile as tile
from concourse import bacc, masks, mybir
from concourse.bass_utils import run_bass_kernel_spmd

AF = mybir.ActivationFunctionType
ALU = mybir.AluOpType
DT = mybir.dt
PM = mybir.MatmulPerfMode

N_CORES = 8
N = 4096          # points per set per batch element
C = 128           # channels (= contraction dim = partition dim)
NB = N // 128     # 32 row blocks of x
UCOLS = 2048      # y columns per PSUM unit (half of PSUM)
NH = N // UCOLS   # 2 column halves
MMN = 512         # moving free dim per matmul (one fp32 PSUM bank)
NT = UCOLS // 128 # 16 transpose tiles per column half
NRES = 3          # fp8 residual rows per squared-norm bake

BETA = 0.75       # softmin sharpness (exp mode)
OFF = 72.0        # PSUM-baked offset row (exact in e4m3): m' = -d2/2 + OFF
REF = 2.0 * OFF   # softmin reference = 144 (exp mode)

_CACHE = {}
LAST_RESULTS = None  # BassKernelResults of the most recent run (for profiling)

DEFAULT_PARTS = "pe,act,dve,exp,dcopy0,sevac"


def _build_program(repeat=1, parts=DEFAULT_PARTS):
    en_act = "act" in parts
    en_dve = "dve" in parts
    en_col = "nocol" not in parts
    en_row = "norow" not in parts
    rowttr = "rowttr" in parts
    en_exp = "exp" in parts
    sevac = "sevac" in parts  # split-unit evac: ACT cols 0:SEA, DVE the rest
    SEA = 1536
    mmn = MMN
    ncopy = 0
    for p in parts.split(","):
        if p.startswith("dcopy"):
            ncopy = int(p[5:])
        if p.startswith("mmn"):
            mmn = int(p[3:])
    # dcopy blocks spread evenly through the 32 row blocks
    dset = set()
    if ncopy:
        step = NB // ncopy
        dset = {step // 2 + k * step for k in range(ncopy)}

    nc = bacc.Bacc(
        "TRN2", debug=False, target_bir_lowering=False, num_devices=N_CORES
    )
    xt_d = nc.dram_tensor("xt", [C, N], DT.float32, kind="ExternalInput").ap()
    yt_d = nc.dram_tensor("yt", [C, N], DT.float32, kind="ExternalInput").ap()
    w1t_d = nc.dram_tensor("w1t", [128, NB], DT.float32, kind="ExternalInput").ap()
    w2t_d = nc.dram_tensor("w2t", [128, NB], DT.float32, kind="ExternalInput").ap()
    hosttail = en_exp and sevac and not ncopy
    if hosttail:
        sadd_d = nc.dram_tensor(
            "sadd_o", [128, NB], DT.float32, kind="ExternalOutput"
        ).ap()
        rmd_d = nc.dram_tensor(
            "rmd_o", [128, NB], DT.float32, kind="ExternalOutput"
        ).ap()
        cmax_d = nc.dram_tensor(
            "cmax_o", [128, NB], DT.float32, kind="ExternalOutput"
        ).ap()
        cmin2_d = nc.dram_tensor(
            "cmin2_o", [128, NB], DT.float32, kind="ExternalOutput"
        ).ap()
    else:
        out_d = nc.dram_tensor(
            "out", [128, 2], DT.float32, kind="ExternalOutput"
        ).ap()

    with tile.TileContext(nc) as tc, ExitStack() as ctx:
        persist = ctx.enter_context(tc.tile_pool(name="persist", bufs=1))
        prep = ctx.enter_context(tc.tile_pool(name="prep", bufs=2))
        d2p = ctx.enter_context(tc.tile_pool(name="d2", bufs=8))
        d2fp = ctx.enter_context(tc.tile_pool(name="d2f", bufs=4))
        psum = ctx.enter_context(tc.tile_pool(name="psum", bufs=2, space="PSUM"))

        # ---------------- inputs ----------------
        xt = persist.tile([C, N], DT.float32)
        yt = persist.tile([C, N], DT.float32)
        nc.sync.dma_start(xt[:], xt_d[:])
        nc.sync.dma_start(yt[:], yt_d[:])
        w1t = persist.tile([128, NB], DT.float32)
        w2t = persist.tile([128, NB], DT.float32)
        nc.sync.dma_start(w1t[:], w1t_d[:])
        nc.sync.dma_start(w2t[:], w2t_d[:])

        # fused fp8 operands: k-tile 0 = channels, k-tile 1 = bake rows
        x8 = persist.tile([C, 2, N], DT.float8e4)
        y8 = persist.tile([C, 2, N], DT.float8e4)
        nc.vector.memset(x8[:], 0.0)
        nc.vector.memset(y8[:], 0.0)
        nc.vector.tensor_copy(x8[:, 0, :], xt[:])
        nc.vector.tensor_copy(y8[:, 0, :], yt[:])

        identity = persist.tile([128, 128], DT.float16)
        masks.make_identity(nc, identity[:])
        identb = persist.tile([128, 128], DT.bfloat16)
        nc.vector.tensor_copy(identb[:], identity[:])

        ones = persist.tile([C, 1], DT.float16)
        nc.gpsimd.memset(ones[:], 1.0)

        # exp-domain (bf16, max-acc) and d2-domain (fp16, min-acc) col accs
        colacc = persist.tile([128, N], DT.bfloat16 if en_exp else DT.float16)
        nc.gpsimd.memset(colacc[:], 0.0 if en_exp else 60000.0)
        colacc2 = None
        if en_exp and sevac:
            colacc2 = persist.tile([128, 2 * (UCOLS - 1536)], DT.float16)
            nc.gpsimd.memset(colacc2[:], 60000.0)
        elif en_exp and ncopy:
            colacc2 = persist.tile([128, N], DT.float16)
            nc.gpsimd.memset(colacc2[:], 60000.0)

        refc = persist.tile([128, 1], DT.float32)
        nc.gpsimd.memset(refc[:], REF)

        rm = persist.tile([128, NB], DT.float32)
        rmh0 = persist.tile([128, NB], DT.float32)
        rmh1 = persist.tile([128, NB], DT.float32)
        se0 = persist.tile([128, NB], DT.float32)
        se1 = persist.tile([128, NB], DT.float32)
        junk = None
        if not en_exp:
            junk = persist.tile([128, N], DT.float16)

        # D-column masks (dcopy mode): dm = 1 on dcopy columns, em = 1 - dm
        dm = em = None
        if ncopy:
            dm = persist.tile([128, NB], DT.float32)
            em = persist.tile([128, NB], DT.float32)
            nc.gpsimd.memset(dm[:], 0.0)
            nc.gpsimd.memset(em[:], 1.0)
            for b in sorted(dset):
                nc.gpsimd.memset(dm[:, b : b + 1], 1.0)
                nc.gpsimd.memset(em[:, b : b + 1], 0.0)

        # ---------------- squared norms -> fp8 residual bake rows ----------
        on8 = persist.tile([1, N], DT.float8e4)
        nc.vector.memset(on8[:], 1.0)
        if en_exp:
            off8 = persist.tile([1, N], DT.float8e4)
            nc.vector.memset(off8[:], OFF)
            nc.sync.dma_start(x8[2 * NRES : 2 * NRES + 1, 1, :], off8[:])
            nc.sync.dma_start(y8[2 * NRES : 2 * NRES + 1, 1, :], on8[:])
        for src, targ, pbase, oth in ((xt, x8, 0, y8), (yt, y8, NRES, x8)):
            sq = prep.tile([C, N], DT.float16, tag="sq", name=f"sq{pbase}")
            nc.scalar.activation(sq[:], src[:], AF.Square)
            v = prep.tile([1, N], DT.float32, tag="v", name=f"v{pbase}")
            for half in range(NH):
                ps = psum.tile(
                    [128, UCOLS], DT.float32, tag="unit", name=f"nps{pbase}{half}"
                )
                for k in range(UCOLS // MMN):
                    c0 = k * MMN
                    nc.tensor.matmul(
                        ps[0:1, c0 : c0 + MMN],
                        ones[:],
                        sq[:, half * UCOLS + c0 : half * UCOLS + c0 + MMN],
                        start=True,
                        stop=True,
                    )
                nc.scalar.activation(
                    v[0:1, half * UCOLS : (half + 1) * UCOLS],
                    ps[0:1, :],
                    AF.Identity,
                    scale=-0.5,
                )
            cur = v
            for k in range(NRES):
                r8 = prep.tile([1, N], DT.float8e4, tag="r8", name=f"r8{pbase}{k}")
                nc.vector.tensor_copy(r8[:], cur[:])
                nc.sync.dma_start(targ[pbase + k : pbase + k + 1, 1, :], r8[:])
                nc.sync.dma_start(oth[pbase + k : pbase + k + 1, 1, :], on8[:])
                if k < NRES - 1:
                    rb = prep.tile([1, N], DT.float32, tag="rb", name=f"rb{pbase}{k}")
                    nc.vector.tensor_copy(rb[:], r8[:])
                    nxt = prep.tile(
                        [1, N], DT.float32, tag="v", name=f"v{pbase}{k}"
                    )
                    nc.vector.tensor_tensor(nxt[:], cur[:], rb[:], ALU.subtract)
                    cur = nxt

        colminT2p = None
        if en_exp and sevac:
            colminT2p = persist.tile([128, NB], DT.float32)
            nc.gpsimd.memset(colminT2p[:], 60000.0)

        if not (en_dve and en_act and en_row):
            nc.gpsimd.memset(rm[:], 1.0)

        unroll = "unroll" in parts
        with (
            tc.For_i(0, repeat, 1)
            if (repeat > 1 and not unroll)
            else nullcontext()
        ):
          for _u in range(repeat if unroll else 1):
            # per-iteration neutral fill for partial-column accumulators
            if en_exp and ncopy:
                nc.gpsimd.memset(se0[:], 1.0)
                nc.gpsimd.memset(se1[:], 1.0)
                nc.gpsimd.memset(rmh0[:], 1.0)
                nc.gpsimd.memset(rmh1[:], 1.0)
            # ---------------- main loop ----------------
            for b in range(NB):
                bcols = slice(b * 128, (b + 1) * 128)
                dvecopy = b in dset and en_dve
                d2fb = None
                if en_exp and sevac and not dvecopy and en_dve:
                    d2fb = d2fp.tile(
                        [128, 2 * (UCOLS - SEA)],
                        DT.float16,
                        tag="d2f",
                        name="d2fb",
                    )
                if not en_exp:
                    if dvecopy:
                        d2 = d2fp.tile([128, N], DT.float16, tag="d2f", name="d2f")
                    else:
                        d2 = d2p.tile([128, N], DT.float16, tag="d2")
                for h in range(NH):
                    hcols = slice(h * UCOLS, (h + 1) * UCOLS)
                    if en_exp:
                        # per-unit tile: no intra-block false deps
                        if dvecopy:
                            d2u = d2fp.tile(
                                [128, UCOLS], DT.float16, tag="d2f", name="d2f"
                            )
                        else:
                            d2u = d2p.tile(
                                [128, SEA if sevac else UCOLS],
                                DT.bfloat16,
                                tag="d2",
                            )
                    ps = psum.tile([128, UCOLS], DT.float32, tag="unit")
                    for k in range(UCOLS // mmn):
                        c0 = k * mmn
                        nc.tensor.matmul(
                            ps[:, c0 : c0 + mmn],
                            x8[:, :, bcols],
                            y8[:, :, h * UCOLS + c0 : h * UCOLS + c0 + mmn],
                            start=True,
                            stop=True,
                            perf_mode=PM.DoubleRow,
                        )
                    rmh = rmh0 if h == 0 else rmh1
                    seh = se0 if h == 0 else se1
                    if en_exp and sevac and not dvecopy:
                        PART = UCOLS - SEA
                        if en_act:
                            nc.scalar.activation(
                                d2u[:, 0:SEA],
                                ps[:, 0:SEA],
                                AF.Exp,
                                scale=2.0 * BETA,
                                accum_out=seh[:, b : b + 1] if en_row else None,
                            )
                        if en_dve:
                            nc.vector.tensor_scalar(
                                d2fb[:, h * PART : (h + 1) * PART],
                                ps[:, SEA:UCOLS],
                                -2.0,
                                None,
                                ALU.mult,
                                ALU.min,
                                accum_out=rmh[:, b : b + 1],
                            )
                            if en_col and en_act:
                                nc.vector.tensor_tensor(
                                    colacc[:, h * UCOLS : h * UCOLS + SEA],
                                    d2u[:, 0:SEA],
                                    colacc[:, h * UCOLS : h * UCOLS + SEA],
                                    ALU.max,
                                )
                        continue
                    if dvecopy and en_exp:
                        nc.vector.tensor_scalar(
                            d2u[:],
                            ps[:],
                            -2.0,
                            None,
                            ALU.mult,
                            ALU.min,
                            accum_out=rmh[:, b : b + 1],
                        )
                        if en_col:
                            nc.vector.tensor_tensor(
                                colacc2[:, hcols],
                                d2u[:],
                                colacc2[:, hcols],
                                ALU.min,
                            )
                    elif dvecopy:
                        # DVE evac (PSUM fp32 -> SBUF fp16 d2, scale -2);
                        # accum is this half's row-min
                        nc.vector.tensor_scalar(
                            d2[:, hcols],
                            ps[:],
                            -2.0,
                            None,
                            ALU.mult,
                            ALU.min,
                            accum_out=rmh[:, b : b + 1],
                        )
                        if en_col:
                            nc.vector.tensor_tensor(
                                colacc2[:, hcols],
                                d2[:, hcols],
                                colacc2[:, hcols],
                                ALU.min,
                            )
                    elif en_act:
                        if en_exp:
                            # evac as exp(-beta*(d2-REF)); accum = softmin sum
                            nc.scalar.activation(
                                d2u[:],
                                ps[:],
                                AF.Exp,
                                scale=2.0 * BETA,
                                accum_out=seh[:, b : b + 1] if en_row else None,
                            )
                            if en_dve and en_col:
                                nc.vector.tensor_tensor(
                                    colacc[:, hcols],
                                    d2u[:],
                                    colacc[:, hcols],
                                    ALU.max,
                                )
                        else:
                            nc.scalar.activation(
                                d2[:, hcols], ps[:], AF.Identity, scale=-2.0
                            )
                            if en_dve and en_col:
                                nc.vector.tensor_tensor(
                                    colacc[:, hcols],
                                    d2[:, hcols],
                                    colacc[:, hcols],
                                    ALU.min,
                                )
                if en_exp and sevac and not dvecopy and en_dve and en_col:
                    nc.vector.tensor_tensor(
                        colacc2[:], d2fb[:], colacc2[:], ALU.min
                    )
                if en_dve and en_act and en_row and not en_exp and not dvecopy:
                    if rowttr:
                        nc.vector.tensor_tensor(
                            junk[:, 0:UCOLS], d2[:, 0:UCOLS], d2[:, UCOLS:N], ALU.min
                        )
                        nc.vector.tensor_tensor(
                            junk[:, 0:1024], junk[:, 0:1024], junk[:, 1024:UCOLS], ALU.min
                        )
                        nc.vector.tensor_tensor(
                            junk[:, 0:512], junk[:, 0:512], junk[:, 512:1024], ALU.min
                        )
                        nc.vector.tensor_reduce(
                            rm[:, b : b + 1],
                            junk[:, 0:512],
                            axis=mybir.AxisListType.X,
                            op=ALU.min,
                        )
                    else:
                        nc.vector.tensor_scalar(
                            junk[:],
                            d2[:],
                            1.0,
                            None,
                            ALU.mult,
                            ALU.min,
                            accum_out=rm[:, b : b + 1],
                        )
                elif en_dve and en_act and en_row and not en_exp and dvecopy:
                    nc.vector.tensor_tensor(
                        rm[:, b : b + 1],
                        rmh0[:, b : b + 1],
                        rmh1[:, b : b + 1],
                        ALU.min,
                    )

            # row softmin sums first: ACT's Ln overlaps DVE's remaining TTs
            lrow = None
            if en_exp:
                sadd = persist.tile([128, NB], DT.float32)
                nc.vector.tensor_tensor(sadd[:], se0[:], se1[:], ALU.add)
                if not hosttail:
                    lrow = persist.tile([128, NB], DT.float32)
                    nc.scalar.activation(lrow[:], sadd[:], AF.Ln)

            # ---------------- column-min tails ----------------
            # exp-domain: transpose colacc (bf16) + strided MAX reduce
            colminT = persist.tile([128, NB], DT.float32)
            for h in range(NH):
                pst = psum.tile(
                    [128, UCOLS],
                    DT.bfloat16 if en_exp else DT.float16,
                    tag="unit",
                    name="pst",
                )
                for t in range(NT):
                    nc.tensor.transpose(
                        pst[:, t * 128 : (t + 1) * 128],
                        colacc[:, h * UCOLS + t * 128 : h * UCOLS + (t + 1) * 128],
                        identb[:] if en_exp else identity[:],
                    )
                nc.vector.tensor_reduce(
                    colminT[:, h * NT : (h + 1) * NT],
                    pst[:].rearrange("p (t c) -> p t c", c=128),
                    axis=mybir.AxisListType.X,
                    op=ALU.max if en_exp else ALU.min,
                )
            colminT2 = None
            if en_exp and sevac:
                colminT2 = colminT2p
                PART = UCOLS - SEA
                NT2 = 2 * PART // 128
                pst2 = psum.tile(
                    [128, 2 * PART], DT.float16, tag="unit", name="pst2"
                )
                for t in range(NT2):
                    nc.tensor.transpose(
                        pst2[:, t * 128 : (t + 1) * 128],
                        colacc2[:, t * 128 : (t + 1) * 128],
                        identity[:],
                    )
                b0 = SEA // 128
                nt_h = PART // 128
                nc.vector.tensor_reduce(
                    colminT2[:, b0 : b0 + nt_h],
                    pst2[:, 0:PART].rearrange("p (t c) -> p t c", c=128),
                    axis=mybir.AxisListType.X,
                    op=ALU.min,
                )
                nc.vector.tensor_reduce(
                    colminT2[:, NT + b0 : NT + b0 + nt_h],
                    pst2[:, PART : 2 * PART].rearrange("p (t c) -> p t c", c=128),
                    axis=mybir.AxisListType.X,
                    op=ALU.min,
                )
            elif en_exp and ncopy:
                colminT2 = persist.tile([128, NB], DT.float32)
                for h in range(NH):
                    pst2 = psum.tile(
                        [128, UCOLS], DT.float16, tag="unit", name="pst2"
                    )
                    for t in range(NT):
                        nc.tensor.transpose(
                            pst2[:, t * 128 : (t + 1) * 128],
                            colacc2[
                                :, h * UCOLS + t * 128 : h * UCOLS + (t + 1) * 128
                            ],
                            identity[:],
                        )
                    nc.vector.tensor_reduce(
                        colminT2[:, h * NT : (h + 1) * NT],
                        pst2[:].rearrange("p (t c) -> p t c", c=128),
                        axis=mybir.AxisListType.X,
                        op=ALU.min,
                    )

            # ---------------- tail ----------------
            if hosttail:
                rmd = persist.tile([128, NB], DT.float32)
                nc.vector.tensor_tensor(rmd[:], rmh0[:], rmh1[:], ALU.min)
                nc.sync.dma_start(sadd_d[:], sadd[:])
                nc.sync.dma_start(rmd_d[:], rmd[:])
                nc.sync.dma_start(cmax_d[:], colminT[:])
                nc.sync.dma_start(cmin2_d[:], colminT2[:])
            elif en_exp:
                # rows: REF - ln(se0+se1)/beta on E columns, min(rmh)+REF on D
                # (sadd and lrow were computed right after the block loop)
                rmd = persist.tile([128, NB], DT.float32)
                if ncopy or sevac:
                    nc.vector.tensor_tensor(rmd[:], rmh0[:], rmh1[:], ALU.min)
                lcol = persist.tile([128, NB], DT.float32)
                nc.scalar.activation(lcol[:], colminT[:], AF.Ln)
                rmexp = persist.tile([128, NB], DT.float32)
                nc.scalar.activation(
                    rmexp[:], lrow[:], AF.Identity, scale=-1.0 / BETA, bias=refc[:]
                )
                colexp = persist.tile([128, NB], DT.float32)
                nc.scalar.activation(
                    colexp[:], lcol[:], AF.Identity, scale=-1.0 / BETA, bias=refc[:]
                )
                colfin = persist.tile([128, NB], DT.float32)
                if sevac:
                    rmds = persist.tile([128, NB], DT.float32)
                    nc.scalar.activation(
                        rmds[:], rmd[:], AF.Identity, bias=refc[:]
                    )
                    cm2s = persist.tile([128, NB], DT.float32)
                    nc.scalar.activation(
                        cm2s[:], colminT2[:], AF.Identity, bias=refc[:]
                    )
                    nc.vector.tensor_tensor(rm[:], rmexp[:], rmds[:], ALU.min)
                    nc.vector.tensor_tensor(
                        colfin[:], colexp[:], cm2s[:], ALU.min
                    )
                elif ncopy:
                    rmds = persist.tile([128, NB], DT.float32)
                    nc.scalar.activation(
                        rmds[:], rmd[:], AF.Identity, bias=refc[:]
                    )
                    cm2s = persist.tile([128, NB], DT.float32)
                    nc.scalar.activation(
                        cm2s[:], colminT2[:], AF.Identity, bias=refc[:]
                    )
                    t1 = persist.tile([128, NB], DT.float32)
                    t2 = persist.tile([128, NB], DT.float32)
                    nc.vector.tensor_mul(t1[:], rmexp[:], em[:])
                    nc.vector.tensor_mul(t2[:], rmds[:], dm[:])
                    nc.vector.tensor_tensor(rm[:], t1[:], t2[:], ALU.add)
                    nc.vector.tensor_tensor(
                        colfin[:], colexp[:], cm2s[:], ALU.min
                    )
                else:
                    nc.vector.tensor_copy(rm[:], rmexp[:])
                    nc.vector.tensor_copy(colfin[:], colexp[:])
            else:
                colfin = colminT

            if not hosttail:
                rowd = persist.tile([128, NB], DT.float32)
                cold = persist.tile([128, NB], DT.float32)
                if en_exp:
                    nc.scalar.activation(rowd[:], rm[:], AF.Sqrt)
                    nc.scalar.activation(cold[:], colfin[:], AF.Sqrt)
                else:
                    rowr = persist.tile([128, NB], DT.float32)
                    nc.scalar.activation(rowr[:], rm[:], AF.Relu)
                    nc.scalar.activation(rowd[:], rowr[:], AF.Sqrt)
                    colr = persist.tile([128, NB], DT.float32)
                    nc.scalar.activation(colr[:], colfin[:], AF.Relu)
                    nc.scalar.activation(cold[:], colr[:], AF.Sqrt)

                junk1 = persist.tile([128, NB], DT.float32)
                outacc = persist.tile([128, 2], DT.float32)
                nc.vector.tensor_mul(junk1[:], rowd[:], w1t[:])
                nc.vector.tensor_reduce(
                    outacc[:, 0:1], junk1[:], axis=mybir.AxisListType.X, op=ALU.add
                )
                junk2 = persist.tile([128, NB], DT.float32)
                nc.vector.tensor_mul(junk2[:], cold[:], w2t[:])
                nc.vector.tensor_reduce(
                    outacc[:, 1:2], junk2[:], axis=mybir.AxisListType.X, op=ALU.add
                )
                nc.sync.dma_start(out_d[:], outacc[:])

    nc.compile()
    return nc


def _get_nc(repeat=1, parts=DEFAULT_PARTS):
    key = ("nc", repeat, parts)
    if key not in _CACHE:
        _CACHE[key] = _build_program(repeat, parts)
    return _CACHE[key]


def _make_in_maps(set1, set2, w1, w2):
    in_maps = []
    for b in range(N_CORES):
        in_maps.append(
            {
                "xt": np.ascontiguousarray(set1[b].T, dtype=np.float32),
                "yt": np.ascontiguousarray(set2[b].T, dtype=np.float32),
                "w1t": np.ascontiguousarray(
                    w1[b].reshape(NB, 128).T, dtype=np.float32
                ),
                "w2t": np.ascontiguousarray(
                    w2[b].reshape(NB, 128).T, dtype=np.float32
                ),
            }
        )
    return in_maps


def kernel(set1, set2, w1, w2):
    global LAST_RESULTS
    set1 = np.asarray(set1, dtype=np.float32)
    set2 = np.asarray(set2, dtype=np.float32)
    w1 = np.asarray(w1, dtype=np.float32)
    w2 = np.asarray(w2, dtype=np.float32)

    nc = _get_nc()
    in_maps = _make_in_maps(set1, set2, w1, w2)
    res = run_bass_kernel_spmd(nc, in_maps, core_ids=list(range(N_CORES)))
    LAST_RESULTS = res

    total = 0.0
    for b, core_out in enumerate(res.results):
        if "out" in core_out:
            total += float(core_out["out"].astype(np.float64).sum())
            continue
        # host tail: softmin -> sqrt -> weighted sums (device shipped raw
        # reductions; layout [p, B] means point index B*128+p)
        sadd = core_out["sadd_o"].astype(np.float64)
        rmd = core_out["rmd_o"].astype(np.float64)
        cmax = core_out["cmax_o"].astype(np.float64)
        cmin2 = core_out["cmin2_o"].astype(np.float64)
        with np.errstate(divide="ignore"):
            rowmin = np.minimum(REF - np.log(sadd) / BETA, rmd + REF)
            colmin = np.minimum(
                np.where(cmax > 0, REF - np.log(np.maximum(cmax, 1e-300)) / BETA, np.inf),
                cmin2 + REF,
            )
        drow = np.sqrt(np.maximum(rowmin, 0.0))
        dcol = np.sqrt(np.maximum(colmin, 0.0))
        w1t = in_maps[b]["w1t"].astype(np.float64)
        w2t = in_maps[b]["w2t"].astype(np.float64)
        total += (w1t * drow).sum() + (w2t * dcol).sum()
    return np.float32(total / 2.0)


# revision 32
# speedup vs baseline: 1.4938x; 1.4938x over previous
"""Weighted Chamfer loss on Trainium2 (8 NeuronCores, batch-parallel).

Problem (per batch element b of 8):
    dist[i, j] = || set1[b, i] - set2[b, j] ||_2            (4096 x 4096, C=128)
    total = (sum_i w1[b,i] * min_j dist + sum_j w2[b,j] * min_i dist) / 2

Sharding: one batch element per NeuronCore (pure data parallel, no
collectives); the 8 per-core partial sums are added on the host.

Per-core pipeline (units are [128 x 2048] fp32 PSUM tiles; 32 row blocks
x 2 column halves):
  PE  : ONE fp8 DoubleRow matmul per 512-col chunk computes
        m = x.y - x2/2 - y2/2 (= -d2/2) over a K=256 contraction:
        k-tile 0 = the 128 channels (e4m3-cast inputs), k-tile 1 = bake
        rows (3 fp8 residual rows per squared-norm + matching ones rows).
  exp mode (default): evacuation carries the row reduction.
    ACT : evacuates E-blocks as exp(-beta*(d2 - REF)) in bf16
          (Exp(scale=2*beta, bias=beta*REF) of the PSUM); its built-in
          sum accumulator emits per-unit softmin sums -> row-min comes
          out of the evac pass for free (rowmin = REF - ln(S)/beta).
    DVE : col pass only = tensor_tensor(max) of the (monotone) exps into
          colacc; for `dcopy` D-blocks DVE instead evacuates from PSUM
          via tensor_scalar(mult -2 -> fp16 d2) whose accum(min) is that
          half's row-min, and folds into a separate fp16 min-colacc
          (rebalances ACT vs DVE).
    Tail: PE transposes of both col accumulators + strided max/min
          reduces; Ln/Relu/Sqrt on ACT; masked merge of the two row/col
          conventions; weighted sums; host adds the 8 partials.
"""

import sys
from contextlib import ExitStack, nullcontext

import numpy as np

for _p in ("/opt/trn_rl_repo",):
    if _p not in sys.path:
        sys.path.insert(0, _p)

import concourse.bass as bass
import concourse.tile as tile
from concourse import bacc, masks, mybir
from concourse.bass_utils import run_bass_kernel_spmd

AF = mybir.ActivationFunctionType
ALU = mybir.AluOpType
DT = mybir.dt
PM = mybir.MatmulPerfMode

N_CORES = 8
N = 4096          # points per set per batch element
C = 128           # channels (= contraction dim = partition dim)
NB = N // 128     # 32 row blocks of x
UCOLS = 2048      # y columns per PSUM unit (half of PSUM)
NH = N // UCOLS   # 2 column halves
MMN = 512         # moving free dim per matmul (one fp32 PSUM bank)
NT = UCOLS // 128 # 16 transpose tiles per column half
NRES = 3          # fp8 residual rows per squared-norm bake

BETA = 0.75       # softmin sharpness (exp mode)
OFF = 72.0        # PSUM-baked offset row (exact in e4m3): m' = -d2/2 + OFF
REF = 2.0 * OFF   # softmin reference = 144 (exp mode)

_CACHE = {}
LAST_RESULTS = None  # BassKernelResults of the most recent run (for profiling)

DEFAULT_PARTS = "pe,act,dve,exp,dcopy0,sevac"


def _build_program(repeat=1, parts=DEFAULT_PARTS):
    en_act = "act" in parts
    en_dve = "dve" in parts
    en_col = "nocol" not in parts
    en_row = "norow" not in parts
    rowttr = "rowttr" in parts
    en_exp = "exp" in parts
    sevac = "sevac" in parts  # split-unit evac: ACT cols 0:SEA, DVE the rest
    SEA = 1536
    mmn = MMN
    ncopy = 0
    for p in parts.split(","):
        if p.startswith("dcopy"):
            ncopy = int(p[5:])
        if p.startswith("mmn"):
            mmn = int(p[3:])
    # dcopy blocks spread evenly through the 32 row blocks
    dset = set()
    if ncopy:
        step = NB // ncopy
        dset = {step // 2 + k * step for k in range(ncopy)}

    nc = bacc.Bacc(
        "TRN2", debug=False, target_bir_lowering=False, num_devices=N_CORES
    )
    xt_d = nc.dram_tensor("xt", [C, N], DT.float32, kind="ExternalInput").ap()
    yt_d = nc.dram_tensor("yt", [C, N], DT.float32, kind="ExternalInput").ap()
    w1t_d = nc.dram_tensor("w1t", [128, NB], DT.float32, kind="ExternalInput").ap()
    w2t_d = nc.dram_tensor("w2t", [128, NB], DT.float32, kind="ExternalInput").ap()
    hosttail = "htail" in parts and en_exp and sevac and not ncopy
    if hosttail:
        sadd_d = nc.dram_tensor(
            "sadd_o", [128, NB], DT.float32, kind="ExternalOutput"
        ).ap()
        rmd_d = nc.dram_tensor(
            "rmd_o", [128, NB], DT.float32, kind="ExternalOutput"
        ).ap()
        cmax_d = nc.dram_tensor(
            "cmax_o", [128, NB], DT.float32, kind="ExternalOutput"
        ).ap()
        cmin2_d = nc.dram_tensor(
            "cmin2_o", [128, NB], DT.float32, kind="ExternalOutput"
        ).ap()
    else:
        out_d = nc.dram_tensor(
            "out", [128, 2], DT.float32, kind="ExternalOutput"
        ).ap()

    with tile.TileContext(nc) as tc, ExitStack() as ctx:
        persist = ctx.enter_context(tc.tile_pool(name="persist", bufs=1))
        prep = ctx.enter_context(tc.tile_pool(name="prep", bufs=2))
        d2p = ctx.enter_context(tc.tile_pool(name="d2", bufs=8))
        d2fp = ctx.enter_context(tc.tile_pool(name="d2f", bufs=4))
        psum = ctx.enter_context(tc.tile_pool(name="psum", bufs=2, space="PSUM"))

        # ---------------- inputs ----------------
        xt = persist.tile([C, N], DT.float32)
        yt = persist.tile([C, N], DT.float32)
        nc.sync.dma_start(xt[:], xt_d[:])
        nc.sync.dma_start(yt[:], yt_d[:])
        w1t = persist.tile([128, NB], DT.float32)
        w2t = persist.tile([128, NB], DT.float32)
        nc.sync.dma_start(w1t[:], w1t_d[:])
        nc.sync.dma_start(w2t[:], w2t_d[:])

        # fused fp8 operands: k-tile 0 = channels, k-tile 1 = bake rows
        x8 = persist.tile([C, 2, N], DT.float8e4)
        y8 = persist.tile([C, 2, N], DT.float8e4)
        nc.vector.memset(x8[:], 0.0)
        nc.vector.memset(y8[:], 0.0)
        nc.vector.tensor_copy(x8[:, 0, :], xt[:])
        nc.vector.tensor_copy(y8[:, 0, :], yt[:])

        identity = persist.tile([128, 128], DT.float16)
        masks.make_identity(nc, identity[:])
        identb = persist.tile([128, 128], DT.bfloat16)
        nc.vector.tensor_copy(identb[:], identity[:])

        ones = persist.tile([C, 1], DT.float16)
        nc.gpsimd.memset(ones[:], 1.0)

        # exp-domain (bf16, max-acc) and d2-domain (fp16, min-acc) col accs
        colacc = persist.tile([128, N], DT.bfloat16 if en_exp else DT.float16)
        nc.gpsimd.memset(colacc[:], 0.0 if en_exp else 60000.0)
        colacc2 = None
        if en_exp and sevac:
            colacc2 = persist.tile([128, 2 * (UCOLS - 1536)], DT.float16)
            nc.gpsimd.memset(colacc2[:], 60000.0)
        elif en_exp and ncopy:
            colacc2 = persist.tile([128, N], DT.float16)
            nc.gpsimd.memset(colacc2[:], 60000.0)

        refc = persist.tile([128, 1], DT.float32)
        nc.gpsimd.memset(refc[:], REF)

        rm = persist.tile([128, NB], DT.float32)
        rmh0 = persist.tile([128, NB], DT.float32)
        rmh1 = persist.tile([128, NB], DT.float32)
        se0 = persist.tile([128, NB], DT.float32)
        se1 = persist.tile([128, NB], DT.float32)
        junk = None
        if not en_exp:
            junk = persist.tile([128, N], DT.float16)

        # D-column masks (dcopy mode): dm = 1 on dcopy columns, em = 1 - dm
        dm = em = None
        if ncopy:
            dm = persist.tile([128, NB], DT.float32)
            em = persist.tile([128, NB], DT.float32)
            nc.gpsimd.memset(dm[:], 0.0)
            nc.gpsimd.memset(em[:], 1.0)
            for b in sorted(dset):
                nc.gpsimd.memset(dm[:, b : b + 1], 1.0)
                nc.gpsimd.memset(em[:, b : b + 1], 0.0)

        # ---------------- squared norms -> fp8 residual bake rows ----------
        on8 = persist.tile([1, N], DT.float8e4)
        nc.vector.memset(on8[:], 1.0)
        if en_exp:
            off8 = persist.tile([1, N], DT.float8e4)
            nc.vector.memset(off8[:], OFF)
            nc.sync.dma_start(x8[2 * NRES : 2 * NRES + 1, 1, :], off8[:])
            nc.sync.dma_start(y8[2 * NRES : 2 * NRES + 1, 1, :], on8[:])
        for src, targ, pbase, oth in ((xt, x8, 0, y8), (yt, y8, NRES, x8)):
            sq = prep.tile([C, N], DT.float16, tag="sq", name=f"sq{pbase}")
            nc.scalar.activation(sq[:], src[:], AF.Square)
            v = prep.tile([1, N], DT.float32, tag="v", name=f"v{pbase}")
            for half in range(NH):
                ps = psum.tile(
                    [128, UCOLS], DT.float32, tag="unit", name=f"nps{pbase}{half}"
                )
                for k in range(UCOLS // MMN):
                    c0 = k * MMN
                    nc.tensor.matmul(
                        ps[0:1, c0 : c0 + MMN],
                        ones[:],
                        sq[:, half * UCOLS + c0 : half * UCOLS + c0 + MMN],
                        start=True,
                        stop=True,
                    )
                nc.scalar.activation(
                    v[0:1, half * UCOLS : (half + 1) * UCOLS],
                    ps[0:1, :],
                    AF.Identity,
                    scale=-0.5,
                )
            cur = v
            for k in range(NRES):
                r8 = prep.tile([1, N], DT.float8e4, tag="r8", name=f"r8{pbase}{k}")
                nc.vector.tensor_copy(r8[:], cur[:])
                nc.sync.dma_start(targ[pbase + k : pbase + k + 1, 1, :], r8[:])
                nc.sync.dma_start(oth[pbase + k : pbase + k + 1, 1, :], on8[:])
                if k < NRES - 1:
                    rb = prep.tile([1, N], DT.float32, tag="rb", name=f"rb{pbase}{k}")
                    nc.vector.tensor_copy(rb[:], r8[:])
                    nxt = prep.tile(
                        [1, N], DT.float32, tag="v", name=f"v{pbase}{k}"
                    )
                    nc.vector.tensor_tensor(nxt[:], cur[:], rb[:], ALU.subtract)
                    cur = nxt

        colminT2p = None
        if en_exp and sevac:
            colminT2p = persist.tile([128, NB], DT.float32)
            nc.gpsimd.memset(colminT2p[:], 60000.0)

        if not (en_dve and en_act and en_row):
            nc.gpsimd.memset(rm[:], 1.0)

        unroll = "unroll" in parts
        with (
            tc.For_i(0, repeat, 1)
            if (repeat > 1 and not unroll)
            else nullcontext()
        ):
          for _u in range(repeat if unroll else 1):
            # per-iteration neutral fill for partial-column accumulators
            if en_exp and ncopy:
                nc.gpsimd.memset(se0[:], 1.0)
                nc.gpsimd.memset(se1[:], 1.0)
                nc.gpsimd.memset(rmh0[:], 1.0)
                nc.gpsimd.memset(rmh1[:], 1.0)
            # ---------------- main loop ----------------
            for b in range(NB):
                bcols = slice(b * 128, (b + 1) * 128)
                dvecopy = b in dset and en_dve
                d2fb = None
                if en_exp and sevac and not dvecopy and en_dve:
                    d2fb = d2fp.tile(
                        [128, 2 * (UCOLS - SEA)],
                        DT.float16,
                        tag="d2f",
                        name="d2fb",
                    )
                if not en_exp:
                    if dvecopy:
                        d2 = d2fp.tile([128, N], DT.float16, tag="d2f", name="d2f")
                    else:
                        d2 = d2p.tile([128, N], DT.float16, tag="d2")
                for h in range(NH):
                    hcols = slice(h * UCOLS, (h + 1) * UCOLS)
                    if en_exp:
                        # per-unit tile: no intra-block false deps
                        if dvecopy:
                            d2u = d2fp.tile(
                                [128, UCOLS], DT.float16, tag="d2f", name="d2f"
                            )
                        else:
                            d2u = d2p.tile(
                                [128, SEA if sevac else UCOLS],
                                DT.bfloat16,
                                tag="d2",
                            )
                    ps = psum.tile([128, UCOLS], DT.float32, tag="unit")
                    for k in range(UCOLS // mmn):
                        c0 = k * mmn
                        nc.tensor.matmul(
                            ps[:, c0 : c0 + mmn],
                            x8[:, :, bcols],
                            y8[:, :, h * UCOLS + c0 : h * UCOLS + c0 + mmn],
                            start=True,
                            stop=True,
                            perf_mode=PM.DoubleRow,
                        )
                    rmh = rmh0 if h == 0 else rmh1
                    seh = se0 if h == 0 else se1
                    if en_exp and sevac and not dvecopy:
                        PART = UCOLS - SEA
                        if en_act:
                            nc.scalar.activation(
                                d2u[:, 0:SEA],
                                ps[:, 0:SEA],
                                AF.Exp,
                                scale=2.0 * BETA,
                                accum_out=seh[:, b : b + 1] if en_row else None,
                            )
                        if en_dve:
                            nc.vector.tensor_scalar(
                                d2fb[:, h * PART : (h + 1) * PART],
                                ps[:, SEA:UCOLS],
                                -2.0,
                                None,
                                ALU.mult,
                                ALU.min,
                                accum_out=rmh[:, b : b + 1],
                            )
                            if en_col and en_act:
                                nc.vector.tensor_tensor(
                                    colacc[:, h * UCOLS : h * UCOLS + SEA],
                                    d2u[:, 0:SEA],
                                    colacc[:, h * UCOLS : h * UCOLS + SEA],
                                    ALU.max,
                                )
                        continue
                    if dvecopy and en_exp:
                        nc.vector.tensor_scalar(
                            d2u[:],
                            ps[:],
                            -2.0,
                            None,
                            ALU.mult,
                            ALU.min,
                            accum_out=rmh[:, b : b + 1],
                        )
                        if en_col:
                            nc.vector.tensor_tensor(
                                colacc2[:, hcols],
                                d2u[:],
                                colacc2[:, hcols],
                                ALU.min,
                            )
                    elif dvecopy:
                        # DVE evac (PSUM fp32 -> SBUF fp16 d2, scale -2);
                        # accum is this half's row-min
                        nc.vector.tensor_scalar(
                            d2[:, hcols],
                            ps[:],
                            -2.0,
                            None,
                            ALU.mult,
                            ALU.min,
                            accum_out=rmh[:, b : b + 1],
                        )
                        if en_col:
                            nc.vector.tensor_tensor(
                                colacc2[:, hcols],
                                d2[:, hcols],
                                colacc2[:, hcols],
                                ALU.min,
                            )
                    elif en_act:
                        if en_exp:
                            # evac as exp(-beta*(d2-REF)); accum = softmin sum
                            nc.scalar.activation(
                                d2u[:],
                                ps[:],
                                AF.Exp,
                                scale=2.0 * BETA,
                                accum_out=seh[:, b : b + 1] if en_row else None,
                            )
                            if en_dve and en_col:
                                nc.vector.tensor_tensor(
                                    colacc[:, hcols],
                                    d2u[:],
                                    colacc[:, hcols],
                                    ALU.max,
                                )
                        else:
                            nc.scalar.activation(
                                d2[:, hcols], ps[:], AF.Identity, scale=-2.0
                            )
                            if en_dve and en_col:
                                nc.vector.tensor_tensor(
                                    colacc[:, hcols],
                                    d2[:, hcols],
                                    colacc[:, hcols],
                                    ALU.min,
                                )
                if en_exp and sevac and not dvecopy and en_dve and en_col:
                    nc.vector.tensor_tensor(
                        colacc2[:], d2fb[:], colacc2[:], ALU.min
                    )
                if en_dve and en_act and en_row and not en_exp and not dvecopy:
                    if rowttr:
                        nc.vector.tensor_tensor(
                            junk[:, 0:UCOLS], d2[:, 0:UCOLS], d2[:, UCOLS:N], ALU.min
                        )
                        nc.vector.tensor_tensor(
                            junk[:, 0:1024], junk[:, 0:1024], junk[:, 1024:UCOLS], ALU.min
                        )
                        nc.vector.tensor_tensor(
                            junk[:, 0:512], junk[:, 0:512], junk[:, 512:1024], ALU.min
                        )
                        nc.vector.tensor_reduce(
                            rm[:, b : b + 1],
                            junk[:, 0:512],
                            axis=mybir.AxisListType.X,
                            op=ALU.min,
                        )
                    else:
                        nc.vector.tensor_scalar(
                            junk[:],
                            d2[:],
                            1.0,
                            None,
                            ALU.mult,
                            ALU.min,
                            accum_out=rm[:, b : b + 1],
                        )
                elif en_dve and en_act and en_row and not en_exp and dvecopy:
                    nc.vector.tensor_tensor(
                        rm[:, b : b + 1],
                        rmh0[:, b : b + 1],
                        rmh1[:, b : b + 1],
                        ALU.min,
                    )

            # row softmin sums first: ACT's Ln overlaps DVE's remaining TTs
            lrow = None
            if en_exp:
                sadd = persist.tile([128, NB], DT.float32)
                nc.vector.tensor_tensor(sadd[:], se0[:], se1[:], ALU.add)
                if not hosttail:
                    lrow = persist.tile([128, NB], DT.float32)
                    nc.scalar.activation(lrow[:], sadd[:], AF.Ln)

            # ---------------- column-min tails ----------------
            # exp-domain: transpose colacc (bf16) + strided MAX reduce
            colminT = persist.tile([128, NB], DT.float32)
            for h in range(NH):
                pst = psum.tile(
                    [128, UCOLS],
                    DT.bfloat16 if en_exp else DT.float16,
                    tag="unit",
                    name="pst",
                )
                for t in range(NT):
                    nc.tensor.transpose(
                        pst[:, t * 128 : (t + 1) * 128],
                        colacc[:, h * UCOLS + t * 128 : h * UCOLS + (t + 1) * 128],
                        identb[:] if en_exp else identity[:],
                    )
                nc.vector.tensor_reduce(
                    colminT[:, h * NT : (h + 1) * NT],
                    pst[:].rearrange("p (t c) -> p t c", c=128),
                    axis=mybir.AxisListType.X,
                    op=ALU.max if en_exp else ALU.min,
                )
            colminT2 = None
            if en_exp and sevac:
                colminT2 = colminT2p
                PART = UCOLS - SEA
                NT2 = 2 * PART // 128
                pst2 = psum.tile(
                    [128, 2 * PART], DT.float16, tag="unit", name="pst2"
                )
                for t in range(NT2):
                    nc.tensor.transpose(
                        pst2[:, t * 128 : (t + 1) * 128],
                        colacc2[:, t * 128 : (t + 1) * 128],
                        identity[:],
                    )
                b0 = SEA // 128
                nt_h = PART // 128
                nc.vector.tensor_reduce(
                    colminT2[:, b0 : b0 + nt_h],
                    pst2[:, 0:PART].rearrange("p (t c) -> p t c", c=128),
                    axis=mybir.AxisListType.X,
                    op=ALU.min,
                )
                nc.vector.tensor_reduce(
                    colminT2[:, NT + b0 : NT + b0 + nt_h],
                    pst2[:, PART : 2 * PART].rearrange("p (t c) -> p t c", c=128),
                    axis=mybir.AxisListType.X,
                    op=ALU.min,
                )
            elif en_exp and ncopy:
                colminT2 = persist.tile([128, NB], DT.float32)
                for h in range(NH):
                    pst2 = psum.tile(
                        [128, UCOLS], DT.float16, tag="unit", name="pst2"
                    )
                    for t in range(NT):
                        nc.tensor.transpose(
                            pst2[:, t * 128 : (t + 1) * 128],
                            colacc2[
                                :, h * UCOLS + t * 128 : h * UCOLS + (t + 1) * 128
                            ],
                            identity[:],
                        )
                    nc.vector.tensor_reduce(
                        colminT2[:, h * NT : (h + 1) * NT],
                        pst2[:].rearrange("p (t c) -> p t c", c=128),
                        axis=mybir.AxisListType.X,
                        op=ALU.min,
                    )

            # ---------------- tail ----------------
            if hosttail:
                rmd = persist.tile([128, NB], DT.float32)
                nc.vector.tensor_tensor(rmd[:], rmh0[:], rmh1[:], ALU.min)
                nc.sync.dma_start(sadd_d[:], sadd[:])
                nc.sync.dma_start(rmd_d[:], rmd[:])
                nc.sync.dma_start(cmax_d[:], colminT[:])
                nc.sync.dma_start(cmin2_d[:], colminT2[:])
            elif en_exp:
                # rows: REF - ln(se0+se1)/beta on E columns, min(rmh)+REF on D
                # (sadd and lrow were computed right after the block loop)
                rmd = persist.tile([128, NB], DT.float32)
                if ncopy or sevac:
                    nc.vector.tensor_tensor(rmd[:], rmh0[:], rmh1[:], ALU.min)
                lcol = persist.tile([128, NB], DT.float32)
                nc.scalar.activation(lcol[:], colminT[:], AF.Ln)
                rmexp = persist.tile([128, NB], DT.float32)
                nc.scalar.activation(
                    rmexp[:], lrow[:], AF.Identity, scale=-1.0 / BETA, bias=refc[:]
                )
                colexp = persist.tile([128, NB], DT.float32)
                nc.scalar.activation(
                    colexp[:], lcol[:], AF.Identity, scale=-1.0 / BETA, bias=refc[:]
                )
                colfin = persist.tile([128, NB], DT.float32)
                if sevac:
                    rmds = persist.tile([128, NB], DT.float32)
                    nc.scalar.activation(
                        rmds[:], rmd[:], AF.Identity, bias=refc[:]
                    )
                    cm2s = persist.tile([128, NB], DT.float32)
                    nc.scalar.activation(
                        cm2s[:], colminT2[:], AF.Identity, bias=refc[:]
                    )
                    nc.vector.tensor_tensor(rm[:], rmexp[:], rmds[:], ALU.min)
                    nc.vector.tensor_tensor(
                        colfin[:], colexp[:], cm2s[:], ALU.min
                    )
                elif ncopy:
                    rmds = persist.tile([128, NB], DT.float32)
                    nc.scalar.activation(
                        rmds[:], rmd[:], AF.Identity, bias=refc[:]
                    )
                    cm2s = persist.tile([128, NB], DT.float32)
                    nc.scalar.activation(
                        cm2s[:], colminT2[:], AF.Identity, bias=refc[:]
                    )
                    t1 = persist.tile([128, NB], DT.float32)
                    t2 = persist.tile([128, NB], DT.float32)
                    nc.vector.tensor_mul(t1[:], rmexp[:], em[:])
                    nc.vector.tensor_mul(t2[:], rmds[:], dm[:])
                    nc.vector.tensor_tensor(rm[:], t1[:], t2[:], ALU.add)
                    nc.vector.tensor_tensor(
                        colfin[:], colexp[:], cm2s[:], ALU.min
                    )
                else:
                    nc.vector.tensor_copy(rm[:], rmexp[:])
                    nc.vector.tensor_copy(colfin[:], colexp[:])
            else:
                colfin = colminT

            if not hosttail:
                rowd = persist.tile([128, NB], DT.float32)
                cold = persist.tile([128, NB], DT.float32)
                if en_exp:
                    nc.scalar.activation(rowd[:], rm[:], AF.Sqrt)
                    nc.scalar.activation(cold[:], colfin[:], AF.Sqrt)
                else:
                    rowr = persist.tile([128, NB], DT.float32)
                    nc.scalar.activation(rowr[:], rm[:], AF.Relu)
                    nc.scalar.activation(rowd[:], rowr[:], AF.Sqrt)
                    colr = persist.tile([128, NB], DT.float32)
                    nc.scalar.activation(colr[:], colfin[:], AF.Relu)
                    nc.scalar.activation(cold[:], colr[:], AF.Sqrt)

                junk1 = persist.tile([128, NB], DT.float32)
                outacc = persist.tile([128, 2], DT.float32)
                nc.vector.tensor_mul(junk1[:], rowd[:], w1t[:])
                nc.vector.tensor_reduce(
                    outacc[:, 0:1], junk1[:], axis=mybir.AxisListType.X, op=ALU.add
                )
                junk2 = persist.tile([128, NB], DT.float32)
                nc.vector.tensor_mul(junk2[:], cold[:], w2t[:])
                nc.vector.tensor_reduce(
                    outacc[:, 1:2], junk2[:], axis=mybir.AxisListType.X, op=ALU.add
                )
                nc.sync.dma_start(out_d[:], outacc[:])

    nc.compile()
    return nc


def _get_nc(repeat=1, parts=DEFAULT_PARTS):
    key = ("nc", repeat, parts)
    if key not in _CACHE:
        _CACHE[key] = _build_program(repeat, parts)
    return _CACHE[key]


def _make_in_maps(set1, set2, w1, w2):
    in_maps = []
    for b in range(N_CORES):
        in_maps.append(
            {
                "xt": np.ascontiguousarray(set1[b].T, dtype=np.float32),
                "yt": np.ascontiguousarray(set2[b].T, dtype=np.float32),
                "w1t": np.ascontiguousarray(
                    w1[b].reshape(NB, 128).T, dtype=np.float32
                ),
                "w2t": np.ascontiguousarray(
                    w2[b].reshape(NB, 128).T, dtype=np.float32
                ),
            }
        )
    return in_maps


def kernel(set1, set2, w1, w2):
    global LAST_RESULTS
    set1 = np.asarray(set1, dtype=np.float32)
    set2 = np.asarray(set2, dtype=np.float32)
    w1 = np.asarray(w1, dtype=np.float32)
    w2 = np.asarray(w2, dtype=np.float32)

    nc = _get_nc()
    in_maps = _make_in_maps(set1, set2, w1, w2)
    res = run_bass_kernel_spmd(nc, in_maps, core_ids=list(range(N_CORES)))
    LAST_RESULTS = res

    total = 0.0
    for b, core_out in enumerate(res.results):
        if "out" in core_out:
            total += float(core_out["out"].astype(np.float64).sum())
            continue
        # host tail: softmin -> sqrt -> weighted sums (device shipped raw
        # reductions; layout [p, B] means point index B*128+p)
        sadd = core_out["sadd_o"].astype(np.float64)
        rmd = core_out["rmd_o"].astype(np.float64)
        cmax = core_out["cmax_o"].astype(np.float64)
        cmin2 = core_out["cmin2_o"].astype(np.float64)
        with np.errstate(divide="ignore"):
            rowmin = np.minimum(REF - np.log(sadd) / BETA, rmd + REF)
            colmin = np.minimum(
                np.where(cmax > 0, REF - np.log(np.maximum(cmax, 1e-300)) / BETA, np.inf),
                cmin2 + REF,
            )
        drow = np.sqrt(np.maximum(rowmin, 0.0))
        dcol = np.sqrt(np.maximum(colmin, 0.0))
        w1t = in_maps[b]["w1t"].astype(np.float64)
        w2t = in_maps[b]["w2t"].astype(np.float64)
        total += (w1t * drow).sum() + (w2t * dcol).sum()
    return np.float32(total / 2.0)
